# revision 2
# baseline (speedup 1.0000x reference)
"""Trainium2 Bass kernel v2 for the dense transformer block (cross-attn + FFN).

Problem: nn_MAB (B=4, nq=nk=1024, D=1024, H=16, HD=64), fp32 in/out.

Sharding: data-parallel, zero collectives. 8 cores = 4 batches x 2
query-halves; each core computes 512 query rows of one batch end-to-end.

v2 changes vs baseline:
  - Host-side key compaction: only keys with mask_y=1 are shipped (padded to
    NT=640, covers the ~512+-30 valid keys with 8-sigma margin); cuts K/V
    projections, scores, exp and attnV by ~37%. Fallback to NT=1024 variant
    if any batch has >640 valid keys (lazily compiled, never hit in practice).
  - bf16 matmul operands everywhere (PSUM + LN stats stay f32): halves DMA
    bytes + SBUF footprint, 2x DVE throughput. rel-err budget 2e-2 >> bf16's
    ~5e-3.
  - Softmax denominator fused into the attnV matmul as a ones-column on the
    V lhsT (M=65); per-query reciprocal is replicated across 64 partitions
    with a K=1 ones matmul instead of per-key-tile denominator matmuls.
  - All 6 weight matrices fully SBUF-resident in bf16, each loaded by one
    fully-contiguous DMA from a host-pre-arranged [128, ...] layout.
  - Tail (O-proj / LN1 / FFN / LN2) processed in two 256-query chunks so the
    LN DVE chains overlap PE matmuls of the other chunk.
"""

import numpy as np
import ml_dtypes

import concourse.bass as bass
import concourse.mybir as mybir
import concourse.tile as tile
from concourse import bacc
from concourse.bass_utils import run_bass_kernel_spmd

F32 = mybir.dt.float32
F32R = mybir.dt.float32r
BF16 = mybir.dt.bfloat16
AF = mybir.ActivationFunctionType

D = 1024          # model dim
P = 128           # partitions
NJ = D // P       # feature tiles (8)
NQ = 512          # queries per core
H = 16
HD = 64
NPAIR = H // 2    # head pairs (8)
NEG = -30000.0    # additive mask for dropped/pad keys
EPS = 1e-5
QC = 256          # tail query-chunk
NQC = NQ // QC    # tail chunks (2)


def build_nc(nkt: int, reps: int = 1) -> bass.Bass:
    """nkt = number of 128-key tiles (5 for the compacted fast path).
    reps > 1 wraps the body in a hardware loop (timing amplification only)."""
    from contextlib import nullcontext
    nt = nkt * P
    nc = bacc.Bacc("TRN2", target_bir_lowering=False, debug=False)

    # ---- DRAM I/O (per-core shards; host prepares exact SBUF layouts) ----
    xt = nc.dram_tensor("xt", [P, NJ * NQ], BF16, kind="ExternalInput")[:]
    yt = nc.dram_tensor("yt", [P, nkt * NJ * P], BF16, kind="ExternalInput")[:]
    wq = nc.dram_tensor("wq", [P, NJ * D], BF16, kind="ExternalInput")[:]
    wk = nc.dram_tensor("wk", [P, NJ * D], BF16, kind="ExternalInput")[:]
    wv = nc.dram_tensor("wv", [P, NJ * D], BF16, kind="ExternalInput")[:]
    wo = nc.dram_tensor("wo", [P, NJ * D], BF16, kind="ExternalInput")[:]
    w1 = nc.dram_tensor("w1", [P, NJ * D], BF16, kind="ExternalInput")[:]
    w2 = nc.dram_tensor("w2", [P, NJ * D], BF16, kind="ExternalInput")[:]
    # one [p, nkt + 6*NJ] tensor: mask bias, then g1/bb1/g2/bb2/b1/b2 blocks
    vecs = nc.dram_tensor("vecs", [P, nkt + 6 * NJ], F32, kind="ExternalInput")[:]
    outt = nc.dram_tensor("outt", [P, NJ * NQ], F32, kind="ExternalOutput")[:]

    with tile.TileContext(nc) as tc, \
         nc.allow_low_precision(reason="bf16 matmul path; tol 2e-2"), \
         (tc.For_i(0, reps) if reps > 1 else nullcontext()), \
         tc.tile_pool(name="persist", bufs=1) as persist, \
         tc.tile_pool(name="psum", bufs=8, space="PSUM") as pp:
        if True:

            def ps_tile(name):
                return pp.tile([P, NQ], F32, tag="ps", name=name)

            # ---- input DMAs, ordered by first use on the SP queue ----
            # yt chunks + wv halves first (V proj), then wk, xt, wq, tail Ws.
            yt_sb = persist.tile([P, nkt, NJ, P], BF16)
            wv_sb = persist.tile([P, NJ, D], BF16)
            wk_sb = persist.tile([P, NJ, D], BF16)
            wq_sb = persist.tile([P, NJ, D], BF16)
            wo_sb = persist.tile([P, NJ, D], BF16)
            w1_sb = persist.tile([P, NJ, D], BF16)
            w2_sb = persist.tile([P, NJ, D], BF16)
            xt_sb = persist.tile([P, NJ, NQ], BF16)

            # ---- small vectors first (mask_sb gates the first exp) ----
            vec_sb = persist.tile([P, nkt + 6 * NJ], F32)
            nc.sync.dma_start(vec_sb, vecs)
            mask_sb = vec_sb[:, 0:nkt]
            g1_sb = vec_sb[:, nkt + 0 * NJ:nkt + 1 * NJ]
            bb1_sb = vec_sb[:, nkt + 1 * NJ:nkt + 2 * NJ]
            g2_sb = vec_sb[:, nkt + 2 * NJ:nkt + 3 * NJ]
            bb2_sb = vec_sb[:, nkt + 3 * NJ:nkt + 4 * NJ]
            b1_sb = vec_sb[:, nkt + 4 * NJ:nkt + 5 * NJ]
            b2_sb = vec_sb[:, nkt + 5 * NJ:nkt + 6 * NJ]

            # big loads, all on the SP queue in first-use order; wv in
            # quarters so the V matmuls start after ~5us of DMA.
            yt_r = yt.rearrange("p (t k m) -> p t k m", t=nkt, k=NJ)
            for t in range(nkt):
                nc.sync.dma_start(yt_sb[:, t], yt_r[:, t])
            wv_r = wv.rearrange("p (k m) -> p k m", k=NJ)
            for ci in range(4):
                nc.sync.dma_start(wv_sb[:, :, ci * 256:(ci + 1) * 256],
                                  wv_r[:, :, ci * 256:(ci + 1) * 256])
            nc.sync.dma_start(wk_sb, wk.rearrange("p (k m) -> p k m", k=NJ))
            nc.sync.dma_start(xt_sb, xt.rearrange("p (j q) -> p j q", j=NJ))
            nc.sync.dma_start(wq_sb, wq.rearrange("p (k m) -> p k m", k=NJ))
            nc.sync.dma_start(wo_sb, wo.rearrange("p (k m) -> p k m", k=NJ))
            nc.sync.dma_start(w1_sb, w1.rearrange("p (k m) -> p k m", k=NJ))
            nc.sync.dma_start(w2_sb, w2.rearrange("p (k m) -> p k m", k=NJ))

            # ---- constants ----
            lnw = persist.tile([P, P], BF16)         # 1/D for LN stat matmuls
            nc.vector.memset(lnw, 1.0 / D)
            cst = persist.tile([P, HD], F32)
            nc.vector.memset(cst, 1.0)
            ones_bc = persist.tile([P, HD], F32R)    # K=1 bcast lhsT (row 64)
            nc.vector.tensor_copy(ones_bc, cst)
            eps_sb = persist.tile([P, 1], F32)
            nc.vector.memset(eps_sb, EPS)

            # attention head outputs, feature-major; tile j rows 0:64 = head
            # 2j+1, rows 64:128 = head 2j (host permutes Wo rows to match).
            outT = persist.tile([P, NJ, NQ], BF16)

            with tc.tile_pool(name="attn", bufs=1) as big:
                # V with an interleaved ones column per head: [keys, h, 65]
                v_sb = big.tile([P, nkt, H, HD + 1], BF16)
                nc.vector.memset(v_sb[:, :, :, HD:HD + 1], 1.0)

                # ---- V = Y @ Wv.T (natural layout), quarter-chunks so the
                # first matmul only waits for yt[0] + a quarter of Wv ----
                for ci in range(4):
                    cs = slice(ci * 256, (ci + 1) * 256)
                    for t in range(nkt):
                        ps = ps_tile("ps_v")
                        for k in range(NJ):
                            nc.tensor.matmul(
                                ps[:, 0:256], yt_sb[:, t, k, :],
                                wv_sb[:, k, cs],
                                start=(k == 0), stop=(k == NJ - 1),
                            )
                        nc.vector.tensor_copy(
                            v_sb[:, t, ci * 4:(ci + 1) * 4, 0:HD], ps[:, 0:256])

                # ---- per head-pair: KT, QT, scoresT, exp, attnV ----
                with tc.tile_pool(name="qk", bufs=2) as qkp, \
                     tc.tile_pool(name="exp", bufs=4) as ep, \
                     tc.tile_pool(name="stage", bufs=3) as stp:
                    deferred = []   # (ps_e, ps_o, rc_e, rc_o, j) of prev pair

                    def flush_deferred():
                        for (pse, pso, rce, rco, jj) in deferred:
                            # replicate reciprocals across 64 partitions
                            bce = ps_tile("ps_bce")
                            nc.tensor.matmul(
                                bce[0:HD, :], ones_bc[HD:HD + 1, :],
                                rce[HD:HD + 1, :], start=True, stop=True,
                                tile_position=(HD, 0),
                            )
                            bco = ps_tile("ps_bco")
                            nc.tensor.matmul(
                                bco[0:HD, :], ones_bc[HD:HD + 1, :],
                                rco[HD:HD + 1, :], start=True, stop=True,
                                tile_position=(HD, 0),
                            )
                            # DVE reads at most one PSUM operand: evict the
                            # replicated reciprocals to SBUF, then divide.
                            rcb_o = stp.tile([P, NQ], BF16, tag="rcb", name="rcb_o")
                            nc.vector.tensor_copy(rcb_o[0:HD, :], bco[0:HD, :])
                            rcb_e = stp.tile([P, NQ], BF16, tag="rcb", name="rcb_e")
                            nc.vector.tensor_copy(rcb_e[0:HD, :], bce[0:HD, :])
                            # odd head -> outT rows 0:64 directly
                            nc.vector.tensor_mul(
                                outT[0:HD, jj, :], pso[0:HD, :], rcb_o[0:HD, :])
                            # even head -> staging, partition-shift to 64:128
                            tmp = stp.tile([P, NQ], BF16, tag="tmp", name="tmp")
                            nc.vector.tensor_mul(
                                tmp[0:HD, :], pse[0:HD, :], rcb_e[0:HD, :])
                            nc.sync.dma_start(outT[HD:P, jj, :], tmp[0:HD, :])
                        deferred.clear()

                    def emit_kq(j):
                        """K^T + Q^T matmuls for pair j -> (kt_j, qt_j)."""
                        ms = slice(j * P, (j + 1) * P)
                        kt_j = qkp.tile([P, nkt * P], BF16, tag="kt", name="kt_j")
                        nch = (nt + NQ - 1) // NQ
                        for c in range(nch):
                            t0 = c * 4
                            tn = min(4, nkt - t0)
                            ps = ps_tile("ps_k")
                            for k in range(NJ):
                                nc.tensor.matmul(
                                    ps[:, 0:tn * P],
                                    wk_sb[:, k, ms],
                                    yt_sb[:, t0:t0 + tn, k, :],
                                    start=(k == 0), stop=(k == NJ - 1),
                                )
                            nc.vector.tensor_copy(
                                kt_j[:, t0 * P:(t0 + tn) * P], ps[:, 0:tn * P])
                        # Q^T m-tile j (Wq pre-scaled by 1/8 on host)
                        qt_j = qkp.tile([P, NQ], BF16, tag="qt", name="qt_j")
                        ps = ps_tile("ps_q")
                        for k in range(NJ):
                            nc.tensor.matmul(
                                ps, wq_sb[:, k, ms], xt_sb[:, k, :],
                                start=(k == 0), stop=(k == NJ - 1),
                            )
                        nc.vector.tensor_copy(qt_j, ps)
                        return kt_j, qt_j

                    def emit_scores(j, kt_j, qt_j):
                        """scoresT + exp for pair j, row-packed on the PE."""
                        exp_e = ep.tile([P, nkt, NQ], BF16, tag="exp", name="exp_e")
                        exp_o = ep.tile([P, nkt, NQ], BF16, tag="exp", name="exp_o")
                        for t in range(nkt):
                            ks = slice(t * P, (t + 1) * P)
                            ps0 = ps_tile("ps_s0")
                            nc.tensor.matmul(
                                ps0, kt_j[0:HD, ks], qt_j[0:HD, :],
                                start=True, stop=True, tile_position=(0, 0),
                            )
                            ps1 = ps_tile("ps_s1")
                            nc.tensor.matmul(
                                ps1, kt_j[HD:P, ks], qt_j[HD:P, :],
                                start=True, stop=True, tile_position=(HD, 0),
                            )
                            nc.scalar.activation(
                                exp_e[:, t, :], ps0, AF.Exp,
                                bias=mask_sb[:, t:t + 1], scale=1.0)
                            nc.scalar.activation(
                                exp_o[:, t, :], ps1, AF.Exp,
                                bias=mask_sb[:, t:t + 1], scale=1.0)
                        return exp_e, exp_o

                    def emit_attnv(j, exp_e, exp_o):
                        """attnV: lhsT = [V_h | ones] (M=65) -> rows 0:64
                        data, row 64 = softmax denominator, for free."""
                        ps_e = ps_tile("ps_ae")
                        ps_o = ps_tile("ps_ao")
                        for t in range(nkt):
                            st, sp = t == 0, t == nkt - 1
                            nc.tensor.matmul(
                                ps_e[0:HD + 1, :], v_sb[:, t, 2 * j, :],
                                exp_e[:, t, :], start=st, stop=sp,
                            )
                        for t in range(nkt):
                            st, sp = t == 0, t == nkt - 1
                            nc.tensor.matmul(
                                ps_o[0:HD + 1, :], v_sb[:, t, 2 * j + 1, :],
                                exp_o[:, t, :], start=st, stop=sp,
                            )
                        # reciprocals (partition 64, f32r) on DVE now; divide
                        # + broadcast run one pair later via flush_deferred
                        rc_e = stp.tile([P, NQ], F32R, tag="rc", name="rc_e")
                        nc.vector.reciprocal(
                            rc_e[HD:HD + 1, :], ps_e[HD:HD + 1, :])
                        rc_o = stp.tile([P, NQ], F32R, tag="rc", name="rc_o")
                        nc.vector.reciprocal(
                            rc_o[HD:HD + 1, :], ps_o[HD:HD + 1, :])
                        deferred.append((ps_e, ps_o, rc_e, rc_o, j))

                    # software pipeline: next pair's K/Q matmuls are issued
                    # between this pair's scores and attnV so PE never waits
                    # on the DVE evictions or the exp activations.
                    kq = emit_kq(0)
                    for j in range(NPAIR):
                        flush_deferred()
                        exps = emit_scores(j, *kq)
                        if j + 1 < NPAIR:
                            kq = emit_kq(j + 1)
                        emit_attnv(j, *exps)
                    flush_deferred()

            # ---- tail: O-proj + LN1 + FFN + LN2, 2 query-chunks ----
            with tc.tile_pool(name="tail", bufs=1) as tl, \
                 tc.tile_pool(name="ln", bufs=8) as lnp:

                x1 = tl.tile([P, NJ, NQ], BF16)
                xsq = tl.tile([P, NJ, NQ], BF16)
                hT = tl.tile([P, NJ, NQ], BF16)
                ff1 = tl.tile([P, NJ, NQ], BF16)
                x2 = tl.tile([P, NJ, NQ], BF16)
                o_sb = tl.tile([P, NJ, NQ], F32)

                def oproj(c):
                    qs = slice(c * QC, (c + 1) * QC)
                    for m in range(NJ):
                        ps = ps_tile("ps_z")
                        for g in range(NJ):
                            nc.tensor.matmul(
                                ps[:, 0:QC], wo_sb[:, g, m * P:(m + 1) * P],
                                outT[:, g, qs],
                                start=(g == 0), stop=(g == NJ - 1))
                        nc.vector.tensor_add(
                            x1[:, m, qs], ps[:, 0:QC], xt_sb[:, m, qs])
                        # eager square so LN1 stats don't wait on ACT later
                        nc.scalar.activation(
                            xsq[:, m, qs], x1[:, m, qs], AF.Square)

                def ln_stats(x_sb, c, name):
                    """mean/E[x^2] matmuls for chunk c (squares already in
                    xsq, computed eagerly at eviction) -> psum pair."""
                    qs = slice(c * QC, (c + 1) * QC)
                    ps_m = ps_tile(name + "_m")
                    for jj in range(NJ):
                        nc.tensor.matmul(
                            ps_m[:, 0:QC], lnw, x_sb[:, jj, qs],
                            start=(jj == 0), stop=(jj == NJ - 1))
                    ps_v = ps_tile(name + "_v")
                    for jj in range(NJ):
                        nc.tensor.matmul(
                            ps_v[:, 0:QC], lnw, xsq[:, jj, qs],
                            start=(jj == 0), stop=(jj == NJ - 1))
                    return ps_m, ps_v

                outt_r = outt.rearrange("p (j q) -> p j q", j=NJ)

                def ln_norm(x_sb, ps_m, ps_v, c, gv, bv, dest, store=False):
                    """DVE/ACT chain: normalize chunk c of x_sb into dest."""
                    qs = slice(c * QC, (c + 1) * QC)
                    mean = lnp.tile([P, QC], F32, tag="lnt", name="mean")
                    nc.vector.tensor_copy(mean, ps_m[:, 0:QC])
                    var = lnp.tile([P, QC], F32, tag="lnt", name="var")
                    nc.vector.tensor_mul(var, mean, mean)
                    nc.vector.tensor_tensor(
                        var, ps_v[:, 0:QC], var, mybir.AluOpType.subtract)
                    sd = lnp.tile([P, QC], F32, tag="lnt", name="sd")
                    nc.scalar.activation(sd, var, AF.Sqrt, bias=eps_sb, scale=1.0)
                    rstd = lnp.tile([P, QC], BF16, tag="lnr", name="rstd")
                    nc.vector.reciprocal(rstd, sd)
                    mrs = lnp.tile([P, QC], BF16, tag="lnr", name="mrs")
                    nc.vector.tensor_mul(mrs, mean, rstd)
                    for jj in range(NJ):
                        t = lnp.tile([P, QC], BF16, tag="lnb", name="t")
                        nc.vector.tensor_mul(t, x_sb[:, jj, qs], rstd)
                        nc.vector.tensor_tensor(
                            t, t, mrs, mybir.AluOpType.subtract)
                        nc.vector.tensor_scalar(
                            dest[:, jj, qs], t,
                            gv[:, jj:jj + 1], bv[:, jj:jj + 1],
                            mybir.AluOpType.mult, mybir.AluOpType.add)
                        if store:
                            nc.sync.dma_start(
                                outt_r[:, jj, qs], dest[:, jj, qs])

                def ffn1(c):
                    qs = slice(c * QC, (c + 1) * QC)
                    for m in range(NJ):
                        ps = ps_tile("ps_f1")
                        for k in range(NJ):
                            nc.tensor.matmul(
                                ps[:, 0:QC], w1_sb[:, k, m * P:(m + 1) * P],
                                hT[:, k, qs],
                                start=(k == 0), stop=(k == NJ - 1))
                        nc.scalar.activation(
                            ff1[:, m, qs], ps[:, 0:QC], AF.Relu,
                            bias=b1_sb[:, m:m + 1], scale=1.0)

                def ffn2(c):
                    qs = slice(c * QC, (c + 1) * QC)
                    for m in range(NJ):
                        ps = ps_tile("ps_f2")
                        for k in range(NJ):
                            nc.tensor.matmul(
                                ps[:, 0:QC], w2_sb[:, k, m * P:(m + 1) * P],
                                ff1[:, k, qs],
                                start=(k == 0), stop=(k == NJ - 1))
                        nc.vector.scalar_tensor_tensor(
                            x2[:, m, qs], ps[:, 0:QC], b2_sb[:, m:m + 1],
                            hT[:, m, qs],
                            op0=mybir.AluOpType.add, op1=mybir.AluOpType.add)
                        nc.scalar.activation(
                            xsq[:, m, qs], x2[:, m, qs], AF.Square)

                # software pipeline over the 2 chunks: each LN's DVE chain is
                # issued right after its stats so no engine-queue inversion,
                # and overlaps the next PE stage.
                oproj(0)
                s1m0, s1v0 = ln_stats(x1, 0, "ln1c0")
                ln_norm(x1, s1m0, s1v0, 0, g1_sb, bb1_sb, hT)
                oproj(1)
                s1m1, s1v1 = ln_stats(x1, 1, "ln1c1")
                ln_norm(x1, s1m1, s1v1, 1, g1_sb, bb1_sb, hT)
                ffn1(0)
                ffn2(0)
                ffn1(1)
                s2m0, s2v0 = ln_stats(x2, 0, "ln2c0")
                ln_norm(x2, s2m0, s2v0, 0, g2_sb, bb2_sb, o_sb, store=True)
                ffn2(1)
                s2m1, s2v1 = ln_stats(x2, 1, "ln2c1")
                ln_norm(x2, s2m1, s2v1, 1, g2_sb, bb2_sb, o_sb, store=True)

    nc.compile()
    return nc


_NC_CACHE: dict = {}


def _get_nc(nkt: int) -> bass.Bass:
    if nkt not in _NC_CACHE:
        _NC_CACHE[nkt] = build_nc(nkt)
    return _NC_CACHE[nkt]


def _bf16(a) -> np.ndarray:
    return np.ascontiguousarray(np.asarray(a, np.float32)).astype(
        ml_dtypes.bfloat16)


def _arrange_w(wt: np.ndarray) -> np.ndarray:
    """[D, D] (in-feat, out-feat) -> [128, NJ*D] with [p, k, m] layout."""
    return np.ascontiguousarray(
        wt.reshape(NJ, P, D).transpose(1, 0, 2).reshape(P, NJ * D))


def _prep_inputs(X, Y, mask_y, Wq, Wk, Wv, Wo, ln1_g, ln1_b, ln2_g, ln2_b,
                 W1, b1, W2, b2):
    X = np.asarray(X, np.float32)
    Y = np.asarray(Y, np.float32)
    mask_y = np.asarray(mask_y)
    B = X.shape[0]

    counts = [int(mask_y[b].sum()) for b in range(B)]
    nkt = 5 if max(counts) <= 5 * P else (max(counts) + P - 1) // P
    nt = nkt * P

    # transposed weights in bf16 (torch Linear: x @ W.T -> lhsT rows = W.T)
    wqt = _bf16(np.asarray(Wq, np.float32).T / np.float32(8.0))
    wkt = _bf16(np.asarray(Wk, np.float32).T)
    wvt = _bf16(np.asarray(Wv, np.float32).T)
    w1t = _bf16(np.asarray(W1, np.float32).T)
    w2t = _bf16(np.asarray(W2, np.float32).T)
    # outT tile j holds head 2j+1 in rows 0:64, head 2j in rows 64:128
    perm = np.empty(D, dtype=np.int64)
    for j in range(NJ):
        perm[j * P:j * P + HD] = (2 * j + 1) * HD + np.arange(HD)
        perm[j * P + HD:(j + 1) * P] = (2 * j) * HD + np.arange(HD)
    wot = _bf16(np.asarray(Wo, np.float32).T[perm])

    vec = lambda v: np.asarray(v, np.float32).reshape(NJ, P).T
    vtail = np.concatenate(
        [vec(v) for v in (ln1_g, ln1_b, ln2_g, ln2_b, b1, b2)], axis=1)
    shared = dict(
        wq=_arrange_w(wqt), wk=_arrange_w(wkt), wv=_arrange_w(wvt),
        wo=_arrange_w(wot), w1=_arrange_w(w1t), w2=_arrange_w(w2t),
    )

    per_batch = {}
    for b in range(B):
        idx = np.flatnonzero(mask_y[b])
        nv = len(idx)
        Yc = np.zeros((nt, D), np.float32)
        bias = np.full(nt, NEG, np.float32)
        if nv == 0:
            bias[0] = 0.0   # zero sentinel key -> attn out = 0/1 = 0
        else:
            Yc[:nv] = Y[b][idx]
            bias[:nv] = 0.0
        # yt layout [p, t, k, m=key-within-tile]: Yc^T[k*128+p, t*128+m]
        ytc = _bf16(Yc.T).reshape(NJ, P, nkt, P).transpose(1, 2, 0, 3)
        per_batch[b] = (
            np.ascontiguousarray(ytc.reshape(P, nkt * NJ * P)),
            np.ascontiguousarray(
                np.concatenate([bias.reshape(nkt, P).T, vtail], axis=1)),
        )

    in_maps = []
    for core in range(8):
        b, half = divmod(core, 2)
        q0 = half * NQ
        m = dict(shared)
        # xt layout [p, j, q]: X^T[j*128+p, q]
        xts = _bf16(X[b, q0:q0 + NQ, :].T)
        m["xt"] = np.ascontiguousarray(
            xts.reshape(NJ, P, NQ).transpose(1, 0, 2).reshape(P, NJ * NQ))
        m["yt"], m["vecs"] = per_batch[b]
        in_maps.append(m)
    return in_maps, nkt


def unpack_output(arrs) -> np.ndarray:
    """arrs: per-core [128, NJ*NQ] f32 -> full [4, 1024, D] output."""
    out = np.empty((4, 1024, D), dtype=np.float32)
    for core in range(8):
        b, half = divmod(core, 2)
        q0 = half * NQ
        a = np.asarray(arrs[core]).reshape(P, NJ, NQ)
        out[b, q0:q0 + NQ, :] = a.transpose(2, 1, 0).reshape(NQ, D)
    return out


def kernel(**inputs) -> np.ndarray:
    in_maps, nkt = _prep_inputs(**inputs)
    res = run_bass_kernel_spmd(_get_nc(nkt), in_maps, core_ids=list(range(8)))
    return unpack_output([res.results[c]["outt"] for c in range(8)])
